# revision 6
# baseline (speedup 1.0000x reference)
"""AnchorAttention Trainium2 kernel (8 NeuronCores, SPMD, no collectives).

Math (per batch): gather anchor rows of hidden_states, LayerNorm, QKV
projections, dense attention among anchors only, out-projection, scatter
back (non-anchor rows of the output are zero; keys are anchors only).

Sharding: core c handles batch c//2 and query-half c%2. The key set is
permutation-invariant, so odd cores receive the anchor rows rolled by
NQ — every core computes attention outputs for "its" first NQ rows.

Device layout (everything transposed, contraction dims on partitions):
  zT   per 512-token chunk: (128, 6, 512)  z = (x-mu)*rstd, d on partitions
  qT   (128, 8, NQ)  per head 128 rows: 96 hd + row96 == 1.0 (mask helper)
  kT   (128, 8, NA)  per head 128 rows: 96 hd + row96 == key-pad mask
  v    (128, T, 8, 97) plain layout, per head 96 dims + ones column
  scores^T (tk, tq) per (head, tk-tile); probs = exp(scale * s)
  avT  (97, NQ) accumulated over tk; row 96 = softmax denominator
  outT (768, NQ) = Wo^T @ (avT / denom) + bo

LayerNorm's affine (ln_g, ln_b) is folded into the weights on the host:
W~ = W * g, bias~ = W @ b + bias.
"""

import numpy as np
import ml_dtypes

import concourse.bass as bass
import concourse.mybir as mybir
import concourse.tile as tile
from concourse import bacc
from concourse.bass_utils import run_bass_kernel_spmd

BF16 = ml_dtypes.bfloat16
F32 = mybir.dt.float32
BF = mybir.dt.bfloat16

B, S, D, H, HD = 4, 2048, 768, 8, 96
J = D // 128          # 6 contraction blocks
EPS = 1e-5
SCALE = 1.0 / np.sqrt(HD)
MASK_NEG = -30000.0   # exp(SCALE * (qk + MASK_NEG)) == 0 in fp32


def _chunks(total, step):
    out = []
    c = 0
    while c < total:
        out.append((c, min(step, total - c)))
        c += step
    return out


def build(NA):
    """Build the per-core Bacc graph for padded anchor count NA."""
    assert NA % 128 == 0
    T = NA // 128
    NQ = NA // 2
    CH = _chunks(NA, 512)          # token chunks (512-wide except tail)
    NCH = len(CH)

    nc = bacc.Bacc("TRN2", target_bir_lowering=False, debug=False, num_devices=8)

    x_ext = nc.dram_tensor("x", [NA, D], BF, kind="ExternalInput").ap()
    wq_ext = nc.dram_tensor("wq", [128, J * 1024], BF, kind="ExternalInput").ap()
    wk_ext = nc.dram_tensor("wk", [128, J * 1024], BF, kind="ExternalInput").ap()
    wv_ext = nc.dram_tensor("wv", [128, J * D], BF, kind="ExternalInput").ap()
    wo_ext = nc.dram_tensor("wo", [96, H * D], BF, kind="ExternalInput").ap()
    bq_ext = nc.dram_tensor("bq", [128, H], F32, kind="ExternalInput").ap()
    bk_ext = nc.dram_tensor("bk", [128, H], F32, kind="ExternalInput").ap()
    bv_ext = nc.dram_tensor("bv", [D], F32, kind="ExternalInput").ap()
    bo_ext = nc.dram_tensor("bo", [128, J], F32, kind="ExternalInput").ap()
    km_ext = nc.dram_tensor("km", [1, NA], BF, kind="ExternalInput").ap()
    out_ext = nc.dram_tensor("out", [D, NQ], F32, kind="ExternalOutput").ap()

    z_dram = nc.dram_tensor("zscratch", [NA, D], BF).ap()

    with tile.TileContext(nc) as tc:
        with (
            tc.tile_pool(name="singles", bufs=1) as singles,
            tc.tile_pool(name="work", bufs=3) as work,
            tc.tile_pool(name="probs", bufs=3) as probs_pool,
        ):
            # ---- weights / constants into SBUF (contiguous per partition).
            # Split per j-block so they interleave with the x-tile loads and
            # the first K-projection can start as soon as block 0 lands.
            wq_sb = singles.tile([128, J, 1024], BF)
            wk_sb = singles.tile([128, J, 1024], BF)
            wv_sb = singles.tile([128, J, D], BF)
            wo_sb = singles.tile([96, H, D], BF)
            wq_v = wq_ext.rearrange("p (j e) -> p j e", j=J)
            wk_v = wk_ext.rearrange("p (j e) -> p j e", j=J)
            wv_v = wv_ext.rearrange("p (j e) -> p j e", j=J)
            for j in range(J):
                nc.sync.dma_start(out=wk_sb[:, j, :], in_=wk_v[:, j, :])
                nc.sync.dma_start(out=wq_sb[:, j, :], in_=wq_v[:, j, :])
                nc.sync.dma_start(out=wv_sb[:, j, :], in_=wv_v[:, j, :])
            nc.sync.dma_start(out=wo_sb, in_=wo_ext)
            bq_sb = singles.tile([128, H], F32)
            nc.sync.dma_start(out=bq_sb, in_=bq_ext)
            bk_sb = singles.tile([128, H], F32)
            nc.sync.dma_start(out=bk_sb, in_=bk_ext)
            bo_sb = singles.tile([128, J], F32)
            nc.sync.dma_start(out=bo_sb, in_=bo_ext)
            bv_sb = singles.tile([128, D], F32)
            bv_bcast = bass.AP(
                tensor=bv_ext.tensor, offset=bv_ext.offset,
                ap=[[0, 128], [1, D]],
            )
            nc.gpsimd.dma_start(out=bv_sb, in_=bv_bcast)

            neg96 = singles.tile([1, 96], BF)
            nc.vector.memset(neg96, -1.0)
            eps_sb = singles.tile([128, 1], F32)
            nc.vector.memset(eps_sb, EPS)

            zT = [singles.tile([128, J, cw], BF, name=f"zT{c}")
                  for c, (c0, cw) in enumerate(CH)]

            def zt_slice(j, c0, cw):
                ci = c0 // 512
                off = c0 % 512
                assert off + cw <= CH[ci][1]
                return zT[ci][:, j, off:off + cw]

            kT = singles.tile([128, H, NA], BF)
            qT = singles.tile([128, H, NQ], BF)
            v_sb = singles.tile([128, T, H, 97], BF)
            avn = singles.tile([96, H, NQ], BF)

            # ones column of v (free slot 96 of each head)
            nc.vector.memset(v_sb[:, :, :, 96:97], 1.0)

            with tc.tile_pool(name="ps_proj", bufs=2, space="PSUM") as ps_proj:
                # Pipeline per 512-token chunk: LN/z -> z_dram -> transpose
                # -> K/Q/V projections for that chunk.
                for ci, (c0, cw) in enumerate(CH):
                    tlo, thi = c0 // 128, (c0 + cw) // 128
                    for i in range(tlo, thi):
                        x_i = work.tile([128, D], BF, tag="x")
                        nc.sync.dma_start(out=x_i, in_=x_ext[i * 128:(i + 1) * 128, :])
                        x_g = x_i.rearrange("p (n f) -> p n f", f=256)
                        stats = work.tile([128, 3, 6], F32, tag="stats")
                        for g in range(3):
                            nc.vector.bn_stats(out=stats[:, g, :], in_=x_g[:, g, :])
                        mv = work.tile([128, 2], F32, tag="mv")
                        nc.vector.bn_aggr(out=mv, in_=stats)
                        sd = work.tile([128, 1], F32, tag="sd")
                        nc.scalar.activation(
                            out=sd, in_=mv[:, 1:2],
                            func=mybir.ActivationFunctionType.Sqrt,
                            bias=eps_sb, scale=1.0,
                        )
                        rstd = work.tile([128, 1], F32, tag="rstd")
                        nc.vector.reciprocal(out=rstd, in_=sd)
                        z_i = work.tile([128, D], BF, tag="z")
                        nc.vector.tensor_scalar(
                            out=z_i, in0=x_i,
                            scalar1=mv[:, 0:1], scalar2=rstd,
                            op0=mybir.AluOpType.subtract, op1=mybir.AluOpType.mult,
                        )
                        nc.sync.dma_start(out=z_dram[i * 128:(i + 1) * 128, :], in_=z_i)

                    for j in range(J):
                        nc.sync.dma_start_transpose(
                            out=zT[ci][:, j, :],
                            in_=z_dram[c0:c0 + cw, j * 128:(j + 1) * 128],
                        )

                    # K projection for this chunk (all heads)
                    for m in range(H):
                        ps = ps_proj.tile([128, cw], F32, tag="proj")
                        for j in range(J):
                            nc.tensor.matmul(
                                ps,
                                lhsT=wk_sb[:, j, m * 128:(m + 1) * 128],
                                rhs=zT[ci][:, j, :cw],
                                start=(j == 0), stop=(j == J - 1),
                            )
                        nc.vector.tensor_scalar_add(
                            out=kT[:, m, c0:c0 + cw], in0=ps,
                            scalar1=bk_sb[:, m:m + 1],
                        )
                    # Q projection for the part of this chunk inside [0, NQ)
                    if c0 < NQ:
                        qw = min(cw, NQ - c0)
                        for m in range(H):
                            ps = ps_proj.tile([128, qw], F32, tag="proj")
                            for j in range(J):
                                nc.tensor.matmul(
                                    ps,
                                    lhsT=wq_sb[:, j, m * 128:(m + 1) * 128],
                                    rhs=zT[ci][:, j, :qw],
                                    start=(j == 0), stop=(j == J - 1),
                                )
                            nc.vector.tensor_scalar_add(
                                out=qT[:, m, c0:c0 + qw], in0=ps,
                                scalar1=bq_sb[:, m:m + 1],
                            )
                    # V projection for this chunk's token tiles
                    for i in range(tlo, thi):
                        for hh in range(4):
                            ps = ps_proj.tile([128, 192], F32, tag="proj")
                            for j in range(J):
                                nc.tensor.matmul(
                                    ps,
                                    lhsT=zt_slice(j, i * 128, 128),
                                    rhs=wv_sb[:, j, hh * 192:(hh + 1) * 192],
                                    start=(j == 0), stop=(j == J - 1),
                                )
                            nc.vector.tensor_tensor(
                                out=v_sb[:, i, 2 * hh:2 * hh + 2, 0:96],
                                in0=ps.rearrange("p (h c) -> p h c", c=96),
                                in1=bv_sb[:, hh * 192:(hh + 1) * 192].rearrange(
                                    "p (h c) -> p h c", c=96),
                                op=mybir.AluOpType.add,
                            )

                # overwrite kT row 96 of every head with the key-pad mask row
                km_bcast = bass.AP(
                    tensor=km_ext.tensor, offset=km_ext.offset,
                    ap=[[0, 1], [0, H], [1, NA]],
                )
                nc.gpsimd.dma_start(out=kT[96:97, :, :], in_=km_bcast)

            # ---- attention (proj psum pool closed; s + av double-buffered) ----
            with (
                tc.tile_pool(name="ps_s", bufs=2, space="PSUM") as ps_s,
                tc.tile_pool(name="ps_av", bufs=2, space="PSUM") as ps_av,
            ):
                for h in range(H):
                    av_ps = ps_av.tile([128, NQ], F32, tag="av")
                    for tk in range(T):
                        s_ps = ps_s.tile([128, NQ], F32, tag="s")
                        for (c0, cw) in _chunks(NQ, 512):
                            nc.tensor.matmul(
                                s_ps[:, c0:c0 + cw],
                                lhsT=kT[:, h, tk * 128:(tk + 1) * 128],
                                rhs=qT[:, h, c0:c0 + cw],
                                start=True, stop=True,
                            )
                        probs = probs_pool.tile([128, NQ], BF, tag="p")
                        nc.scalar.activation(
                            out=probs, in_=s_ps,
                            func=mybir.ActivationFunctionType.Exp,
                            scale=float(SCALE),
                        )
                        for (c0, cw) in _chunks(NQ, 512):
                            nc.tensor.matmul(
                                av_ps[0:97, c0:c0 + cw],
                                lhsT=v_sb[:, tk, h, :],
                                rhs=probs[:, c0:c0 + cw],
                                start=(tk == 0), stop=(tk == T - 1),
                                skip_group_check=True,
                            )
                    # normalize: avn = avT[0:96] / avT[96].  1/d computed as
                    # exp(-ln d): ln on ACT (1 cyc/elem) beats DVE reciprocal
                    # (8 cyc/elem); the negation rides the broadcast matmul
                    # (lhsT == -1), and the exp replaces the PSUM->SBUF copy.
                    logd = work.tile([1, NQ], BF, tag="logd")
                    nc.scalar.activation(
                        out=logd, in_=av_ps[96:97, :],
                        func=mybir.ActivationFunctionType.Ln,
                    )
                    bc_ps = ps_s.tile([96, NQ], F32, tag="s")
                    for (c0, cw) in _chunks(NQ, 512):
                        nc.tensor.matmul(
                            bc_ps[:, c0:c0 + cw],
                            lhsT=neg96,
                            rhs=logd[:, c0:c0 + cw],
                            start=True, stop=True,
                        )
                    bc_sb = work.tile([96, NQ], F32, tag="bc")
                    nc.scalar.activation(
                        out=bc_sb, in_=bc_ps,
                        func=mybir.ActivationFunctionType.Exp,
                    )
                    nc.vector.tensor_tensor(
                        out=avn[:, h, :], in0=av_ps[0:96, :], in1=bc_sb,
                        op=mybir.AluOpType.mult,
                    )

            # ---- out projection ----
            with tc.tile_pool(name="ps_o", bufs=2, space="PSUM") as ps_o:
                for m in range(J):
                    for (c0, cw) in _chunks(NQ, 512):
                        o_ps = ps_o.tile([128, cw], F32, tag="o")
                        for h in range(H):
                            nc.tensor.matmul(
                                o_ps,
                                lhsT=wo_sb[:, h, m * 128:(m + 1) * 128],
                                rhs=avn[:, h, c0:c0 + cw],
                                start=(h == 0), stop=(h == H - 1),
                            )
                        o_sb = work.tile([128, cw], F32, tag="osb")
                        nc.vector.tensor_scalar_add(
                            out=o_sb, in0=o_ps, scalar1=bo_sb[:, m:m + 1],
                        )
                        nc.sync.dma_start(
                            out=out_ext[m * 128:(m + 1) * 128, c0:c0 + cw], in_=o_sb,
                        )

    nc.compile()
    return nc


_CACHE = {}


def _prep_weights(ln_g, ln_b, Wq, bq, Wk, bk, Wv, bv, Wo, bo):
    def pad_head_T(W):
        # (W * g).T padded per head 96 -> 128 cols, then SBUF layout
        # (128, J, 1024): [p, j, e] = WT[j*128+p, e]
        WT = (W * ln_g[None, :]).T.astype(np.float32)
        Wp = np.zeros((D, H, 128), np.float32)
        Wp[:, :, :96] = WT.reshape(D, H, 96)
        Wp = Wp.reshape(J, 128, H * 128).transpose(1, 0, 2)   # (128, J, 1024)
        return np.ascontiguousarray(Wp.reshape(128, J * 1024)).astype(BF16)

    def plain_T(W):
        WT = (W * ln_g[None, :]).T.astype(np.float32)         # (768, 768)
        Wp = WT.reshape(J, 128, D).transpose(1, 0, 2)          # (128, J, 768)
        return np.ascontiguousarray(Wp.reshape(128, J * D)).astype(BF16)

    def pad_bias(bb, ones_row):
        bp = np.zeros((H, 128), np.float32)
        bp[:, :96] = bb.reshape(H, 96)
        if ones_row:
            bp[:, 96] = 1.0
        return np.ascontiguousarray(bp.T).astype(np.float32)   # (128, H)

    return {
        "wq": pad_head_T(Wq),
        "wk": pad_head_T(Wk),
        "wv": plain_T(Wv),
        "wo": np.ascontiguousarray(
            Wo.T.reshape(H, 96, D).transpose(1, 0, 2).reshape(96, H * D)
        ).astype(BF16),
        "bq": pad_bias(Wq @ ln_b + bq, True),
        "bk": pad_bias(Wk @ ln_b + bk, False),
        "bv": (Wv @ ln_b + bv).astype(np.float32),
        "bo": np.ascontiguousarray(bo.reshape(J, 128).T).astype(np.float32),
    }


def _make_in_maps(hidden_states, idx, NA, wmaps):
    NQ = NA // 2
    in_maps = []
    for c in range(8):
        b, half = c // 2, c % 2
        nb = len(idx[b])
        xg = np.zeros((NA, D), np.float32)
        xg[:nb] = hidden_states[b][idx[b]]
        km = np.zeros((NA,), np.float32)
        km[nb:] = MASK_NEG
        if half:
            xg = np.roll(xg, -NQ, axis=0)
            km = np.roll(km, -NQ)
        in_maps.append({
            "x": xg.astype(BF16),
            "km": km.reshape(1, NA).astype(BF16),
            **wmaps,
        })
    return in_maps


def kernel(hidden_states, anchor_mask, ln_g, ln_b,
           Wq, bq, Wk, bk, Wv, bv, Wo, bo):
    hidden_states = np.asarray(hidden_states, dtype=np.float32)
    anchor_mask = np.asarray(anchor_mask).astype(bool)
    args = [np.asarray(a, dtype=np.float32)
            for a in (ln_g, ln_b, Wq, bq, Wk, bk, Wv, bv, Wo, bo)]

    idx = [np.nonzero(anchor_mask[b])[0] for b in range(B)]
    max_nb = max(len(i) for i in idx)
    NA = max(256, ((max_nb + 127) // 128) * 128)
    NQ = NA // 2

    if NA not in _CACHE:
        _CACHE[NA] = build(NA)
    nc = _CACHE[NA]

    wmaps = _prep_weights(*args)
    in_maps = _make_in_maps(hidden_states, idx, NA, wmaps)

    res = run_bass_kernel_spmd(nc, in_maps, core_ids=list(range(8)))

    out = np.zeros((B, S, D), np.float32)
    for c in range(8):
        b, half = c // 2, c % 2
        nb = len(idx[b])
        oT = res.results[c]["out"]  # (768, NQ)
        qtok = (np.arange(NQ) + half * NQ) % NA
        valid = qtok < nb
        out[b, idx[b][qtok[valid]]] = oT.T[valid]
    return out


# revision 9
# speedup vs baseline: 1.1696x; 1.1696x over previous
"""AnchorAttention Trainium2 kernel (8 NeuronCores, SPMD, no collectives).

Math (per batch): gather anchor rows of hidden_states, LayerNorm, QKV
projections, dense attention among anchors only, out-projection, scatter
back (non-anchor rows of the output are zero; keys are anchors only).

Sharding: core c handles batch c//2 and query-half c%2. The key set is
permutation-invariant, so odd cores receive the anchor rows rolled by
NQ — every core computes attention outputs for "its" first NQ rows.

Device layout (everything transposed, contraction dims on partitions):
  zT   per 512-token chunk: (128, 6, 512)  z = (x-mu)*rstd, d on partitions
  qT   (128, 8, NQ)  per head 128 rows: 96 hd + row96 == 1.0 (mask helper)
  kT   (128, 8, NA)  per head 128 rows: 96 hd + row96 == key-pad mask
  v    (128, T, 8, 97) plain layout, per head 96 dims + ones column
  scores^T (tk, tq) per (head, tk-tile); probs = exp(scale * s)
  avT  (97, NQ) accumulated over tk; row 96 = softmax denominator
  outT (768, NQ) = Wo^T @ (avT / denom) + bo

LayerNorm's affine (ln_g, ln_b) is folded into the weights on the host:
W~ = W * g, bias~ = W @ b + bias.
"""

import numpy as np
import ml_dtypes

import concourse.bass as bass
import concourse.mybir as mybir
import concourse.tile as tile
from concourse import bacc
from concourse.bass_utils import run_bass_kernel_spmd

BF16 = ml_dtypes.bfloat16
F32 = mybir.dt.float32
BF = mybir.dt.bfloat16

B, S, D, H, HD = 4, 2048, 768, 8, 96
J = D // 128          # 6 contraction blocks
EPS = 1e-5
SCALE = 1.0 / np.sqrt(HD)
MASK_NEG = -30000.0   # exp(SCALE * (qk + MASK_NEG)) == 0 in fp32


def _chunks(total, step):
    out = []
    c = 0
    while c < total:
        out.append((c, min(step, total - c)))
        c += step
    return out


def build(NA):
    """Build the per-core Bacc graph for padded anchor count NA."""
    assert NA % 128 == 0
    T = NA // 128
    NQ = NA // 2
    CH = _chunks(NA, 512)          # token chunks (512-wide except tail)
    NCH = len(CH)

    nc = bacc.Bacc("TRN2", target_bir_lowering=False, debug=False, num_devices=8)

    x_ext = nc.dram_tensor("x", [NA, D], BF, kind="ExternalInput").ap()
    wq_ext = nc.dram_tensor("wq", [128, J * 1024], BF, kind="ExternalInput").ap()
    wk_ext = nc.dram_tensor("wk", [128, J * 1024], BF, kind="ExternalInput").ap()
    wv_ext = nc.dram_tensor("wv", [128, J * D], BF, kind="ExternalInput").ap()
    wo_ext = nc.dram_tensor("wo", [96, H * D], BF, kind="ExternalInput").ap()
    bq_ext = nc.dram_tensor("bq", [128, H], F32, kind="ExternalInput").ap()
    bk_ext = nc.dram_tensor("bk", [128, H], F32, kind="ExternalInput").ap()
    bv_ext = nc.dram_tensor("bv", [D], F32, kind="ExternalInput").ap()
    bo_ext = nc.dram_tensor("bo", [128, J], F32, kind="ExternalInput").ap()
    km_ext = nc.dram_tensor("km", [1, NA], BF, kind="ExternalInput").ap()
    out_ext = nc.dram_tensor("out", [D, NQ], F32, kind="ExternalOutput").ap()

    z_dram = nc.dram_tensor("zscratch", [NA, D], BF).ap()

    with tile.TileContext(nc) as tc:
        with (
            tc.tile_pool(name="singles", bufs=1) as singles,
            tc.tile_pool(name="work", bufs=3) as work,
            tc.tile_pool(name="probs", bufs=3) as probs_pool,
        ):
            # ---- weights / constants into SBUF (contiguous per partition).
            # Split per j-block so they interleave with the x-tile loads and
            # the first K-projection can start as soon as block 0 lands.
            wq_sb = singles.tile([128, J, 1024], BF)
            wk_sb = singles.tile([128, J, 1024], BF)
            wv_sb = singles.tile([128, J, D], BF)
            wo_sb = singles.tile([96, H, D], BF)
            wq_v = wq_ext.rearrange("p (j e) -> p j e", j=J)
            wk_v = wk_ext.rearrange("p (j e) -> p j e", j=J)
            wv_v = wv_ext.rearrange("p (j e) -> p j e", j=J)
            for j in range(J):
                nc.sync.dma_start(out=wk_sb[:, j, :], in_=wk_v[:, j, :])
                nc.sync.dma_start(out=wq_sb[:, j, :], in_=wq_v[:, j, :])
                nc.sync.dma_start(out=wv_sb[:, j, :], in_=wv_v[:, j, :])
            nc.sync.dma_start(out=wo_sb, in_=wo_ext)
            bq_sb = singles.tile([128, H], F32)
            nc.sync.dma_start(out=bq_sb, in_=bq_ext)
            bk_sb = singles.tile([128, H], F32)
            nc.sync.dma_start(out=bk_sb, in_=bk_ext)
            bo_sb = singles.tile([128, J], F32)
            nc.sync.dma_start(out=bo_sb, in_=bo_ext)
            bv_sb = singles.tile([128, D], F32)
            bv_bcast = bass.AP(
                tensor=bv_ext.tensor, offset=bv_ext.offset,
                ap=[[0, 128], [1, D]],
            )
            nc.gpsimd.dma_start(out=bv_sb, in_=bv_bcast)

            ones96 = singles.tile([1, 96], BF)
            nc.vector.memset(ones96, 1.0)
            eps_sb = singles.tile([128, 1], F32)
            nc.vector.memset(eps_sb, EPS)

            zT = [singles.tile([128, J, cw], BF, name=f"zT{c}")
                  for c, (c0, cw) in enumerate(CH)]

            def zt_slice(j, c0, cw):
                ci = c0 // 512
                off = c0 % 512
                assert off + cw <= CH[ci][1]
                return zT[ci][:, j, off:off + cw]

            kT = singles.tile([128, H, NA], BF)
            qT = singles.tile([128, H, NQ], BF)
            v_sb = singles.tile([128, T, H, 97], BF)
            avn = singles.tile([96, H, NQ], BF)

            # ones column of v (free slot 96 of each head)
            nc.vector.memset(v_sb[:, :, :, 96:97], 1.0)

            with tc.tile_pool(name="ps_proj", bufs=2, space="PSUM") as ps_proj:
                # Pipeline per 512-token chunk: LN/z -> z_dram -> transpose
                # -> K/Q/V projections for that chunk.
                for ci, (c0, cw) in enumerate(CH):
                    tlo, thi = c0 // 128, (c0 + cw) // 128
                    for i in range(tlo, thi):
                        x_i = work.tile([128, D], BF, tag="x")
                        nc.sync.dma_start(out=x_i, in_=x_ext[i * 128:(i + 1) * 128, :])
                        x_g = x_i.rearrange("p (n f) -> p n f", f=256)
                        stats = work.tile([128, 3, 6], F32, tag="stats")
                        for g in range(3):
                            nc.vector.bn_stats(out=stats[:, g, :], in_=x_g[:, g, :])
                        mv = work.tile([128, 2], F32, tag="mv")
                        nc.vector.bn_aggr(out=mv, in_=stats)
                        sd = work.tile([128, 1], F32, tag="sd")
                        nc.scalar.activation(
                            out=sd, in_=mv[:, 1:2],
                            func=mybir.ActivationFunctionType.Sqrt,
                            bias=eps_sb, scale=1.0,
                        )
                        rstd = work.tile([128, 1], F32, tag="rstd")
                        nc.vector.reciprocal(out=rstd, in_=sd)
                        z_i = work.tile([128, D], BF, tag="z")
                        nc.vector.tensor_scalar(
                            out=z_i, in0=x_i,
                            scalar1=mv[:, 0:1], scalar2=rstd,
                            op0=mybir.AluOpType.subtract, op1=mybir.AluOpType.mult,
                        )
                        nc.sync.dma_start(out=z_dram[i * 128:(i + 1) * 128, :], in_=z_i)

                    for j in range(J):
                        nc.sync.dma_start_transpose(
                            out=zT[ci][:, j, :],
                            in_=z_dram[c0:c0 + cw, j * 128:(j + 1) * 128],
                        )

                    # K projection for this chunk (all heads)
                    for m in range(H):
                        ps = ps_proj.tile([128, cw], F32, tag="proj")
                        for j in range(J):
                            nc.tensor.matmul(
                                ps,
                                lhsT=wk_sb[:, j, m * 128:(m + 1) * 128],
                                rhs=zT[ci][:, j, :cw],
                                start=(j == 0), stop=(j == J - 1),
                            )
                        nc.vector.tensor_scalar_add(
                            out=kT[:, m, c0:c0 + cw], in0=ps,
                            scalar1=bk_sb[:, m:m + 1],
                        )
                    # Q projection for the part of this chunk inside [0, NQ)
                    if c0 < NQ:
                        qw = min(cw, NQ - c0)
                        for m in range(H):
                            ps = ps_proj.tile([128, qw], F32, tag="proj")
                            for j in range(J):
                                nc.tensor.matmul(
                                    ps,
                                    lhsT=wq_sb[:, j, m * 128:(m + 1) * 128],
                                    rhs=zT[ci][:, j, :qw],
                                    start=(j == 0), stop=(j == J - 1),
                                )
                            nc.vector.tensor_scalar_add(
                                out=qT[:, m, c0:c0 + qw], in0=ps,
                                scalar1=bq_sb[:, m:m + 1],
                            )
                    # V projection for this chunk's token tiles
                    for i in range(tlo, thi):
                        for hh in range(4):
                            ps = ps_proj.tile([128, 192], F32, tag="proj")
                            for j in range(J):
                                nc.tensor.matmul(
                                    ps,
                                    lhsT=zt_slice(j, i * 128, 128),
                                    rhs=wv_sb[:, j, hh * 192:(hh + 1) * 192],
                                    start=(j == 0), stop=(j == J - 1),
                                )
                            nc.vector.tensor_tensor(
                                out=v_sb[:, i, 2 * hh:2 * hh + 2, 0:96],
                                in0=ps.rearrange("p (h c) -> p h c", c=96),
                                in1=bv_sb[:, hh * 192:(hh + 1) * 192].rearrange(
                                    "p (h c) -> p h c", c=96),
                                op=mybir.AluOpType.add,
                            )

                # overwrite kT row 96 of every head with the key-pad mask row
                km_bcast = bass.AP(
                    tensor=km_ext.tensor, offset=km_ext.offset,
                    ap=[[0, 1], [0, H], [1, NA]],
                )
                nc.gpsimd.dma_start(out=kT[96:97, :, :], in_=km_bcast)

            # ---- attention (proj psum pool closed; s + av double-buffered) ----
            with (
                tc.tile_pool(name="ps_s", bufs=2, space="PSUM") as ps_s,
                tc.tile_pool(name="ps_av", bufs=2, space="PSUM") as ps_av,
            ):
                for h in range(H):
                    av_ps = ps_av.tile([128, NQ], F32, tag="av")
                    for tk in range(T):
                        s_ps = ps_s.tile([128, NQ], F32, tag="s")
                        for (c0, cw) in _chunks(NQ, 512):
                            nc.tensor.matmul(
                                s_ps[:, c0:c0 + cw],
                                lhsT=kT[:, h, tk * 128:(tk + 1) * 128],
                                rhs=qT[:, h, c0:c0 + cw],
                                start=True, stop=True,
                            )
                        probs = probs_pool.tile([128, NQ], BF, tag="p")
                        nc.scalar.activation(
                            out=probs, in_=s_ps,
                            func=mybir.ActivationFunctionType.Exp,
                            scale=float(SCALE),
                        )
                        for (c0, cw) in _chunks(NQ, 512):
                            nc.tensor.matmul(
                                av_ps[0:97, c0:c0 + cw],
                                lhsT=v_sb[:, tk, h, :],
                                rhs=probs[:, c0:c0 + cw],
                                start=(tk == 0), stop=(tk == T - 1),
                                skip_group_check=True,
                            )
                    # normalize: avn = avT[0:96] * (1 / avT[96]) broadcast.
                    # reciprocal_approx_fast (18-bit) is ~5x the plain DVE
                    # reciprocal; denominators are >= 1 so it's in-range.
                    d_sb = work.tile([1, NQ], F32, tag="dsb")
                    nc.vector.tensor_copy(out=d_sb, in_=av_ps[96:97, :])
                    rec32 = work.tile([1, NQ], F32, tag="rec32")
                    nc.vector.reciprocal_approx_fast(out=rec32, in_=d_sb)
                    recip_bf = work.tile([1, NQ], BF, tag="recipbf")
                    nc.vector.tensor_copy(out=recip_bf, in_=rec32)
                    bc_ps = ps_s.tile([96, NQ], F32, tag="s")
                    for (c0, cw) in _chunks(NQ, 512):
                        nc.tensor.matmul(
                            bc_ps[:, c0:c0 + cw],
                            lhsT=ones96,
                            rhs=recip_bf[:, c0:c0 + cw],
                            start=True, stop=True,
                        )
                    bc_sb = work.tile([96, NQ], F32, tag="bc")
                    nc.scalar.activation(
                        out=bc_sb, in_=bc_ps,
                        func=mybir.ActivationFunctionType.Copy,
                    )
                    nc.vector.tensor_tensor(
                        out=avn[:, h, :], in0=av_ps[0:96, :], in1=bc_sb,
                        op=mybir.AluOpType.mult,
                    )

            # ---- out projection ----
            with tc.tile_pool(name="ps_o", bufs=2, space="PSUM") as ps_o:
                for m in range(J):
                    for (c0, cw) in _chunks(NQ, 512):
                        o_ps = ps_o.tile([128, cw], F32, tag="o")
                        for h in range(H):
                            nc.tensor.matmul(
                                o_ps,
                                lhsT=wo_sb[:, h, m * 128:(m + 1) * 128],
                                rhs=avn[:, h, c0:c0 + cw],
                                start=(h == 0), stop=(h == H - 1),
                            )
                        o_sb = work.tile([128, cw], F32, tag="osb")
                        nc.vector.tensor_scalar_add(
                            out=o_sb, in0=o_ps, scalar1=bo_sb[:, m:m + 1],
                        )
                        nc.sync.dma_start(
                            out=out_ext[m * 128:(m + 1) * 128, c0:c0 + cw], in_=o_sb,
                        )

    nc.compile()
    return nc


_CACHE = {}


def _prep_weights(ln_g, ln_b, Wq, bq, Wk, bk, Wv, bv, Wo, bo):
    def pad_head_T(W):
        # (W * g).T padded per head 96 -> 128 cols, then SBUF layout
        # (128, J, 1024): [p, j, e] = WT[j*128+p, e]
        WT = (W * ln_g[None, :]).T.astype(np.float32)
        Wp = np.zeros((D, H, 128), np.float32)
        Wp[:, :, :96] = WT.reshape(D, H, 96)
        Wp = Wp.reshape(J, 128, H * 128).transpose(1, 0, 2)   # (128, J, 1024)
        return np.ascontiguousarray(Wp.reshape(128, J * 1024)).astype(BF16)

    def plain_T(W):
        WT = (W * ln_g[None, :]).T.astype(np.float32)         # (768, 768)
        Wp = WT.reshape(J, 128, D).transpose(1, 0, 2)          # (128, J, 768)
        return np.ascontiguousarray(Wp.reshape(128, J * D)).astype(BF16)

    def pad_bias(bb, ones_row):
        bp = np.zeros((H, 128), np.float32)
        bp[:, :96] = bb.reshape(H, 96)
        if ones_row:
            bp[:, 96] = 1.0
        return np.ascontiguousarray(bp.T).astype(np.float32)   # (128, H)

    return {
        "wq": pad_head_T(Wq),
        "wk": pad_head_T(Wk),
        "wv": plain_T(Wv),
        "wo": np.ascontiguousarray(
            Wo.T.reshape(H, 96, D).transpose(1, 0, 2).reshape(96, H * D)
        ).astype(BF16),
        "bq": pad_bias(Wq @ ln_b + bq, True),
        "bk": pad_bias(Wk @ ln_b + bk, False),
        "bv": (Wv @ ln_b + bv).astype(np.float32),
        "bo": np.ascontiguousarray(bo.reshape(J, 128).T).astype(np.float32),
    }


def _make_in_maps(hidden_states, idx, NA, wmaps):
    NQ = NA // 2
    in_maps = []
    for c in range(8):
        b, half = c // 2, c % 2
        nb = len(idx[b])
        xg = np.zeros((NA, D), np.float32)
        xg[:nb] = hidden_states[b][idx[b]]
        km = np.zeros((NA,), np.float32)
        km[nb:] = MASK_NEG
        if half:
            xg = np.roll(xg, -NQ, axis=0)
            km = np.roll(km, -NQ)
        in_maps.append({
            "x": xg.astype(BF16),
            "km": km.reshape(1, NA).astype(BF16),
            **wmaps,
        })
    return in_maps


def kernel(hidden_states, anchor_mask, ln_g, ln_b,
           Wq, bq, Wk, bk, Wv, bv, Wo, bo):
    hidden_states = np.asarray(hidden_states, dtype=np.float32)
    anchor_mask = np.asarray(anchor_mask).astype(bool)
    args = [np.asarray(a, dtype=np.float32)
            for a in (ln_g, ln_b, Wq, bq, Wk, bk, Wv, bv, Wo, bo)]

    idx = [np.nonzero(anchor_mask[b])[0] for b in range(B)]
    max_nb = max(len(i) for i in idx)
    NA = max(256, ((max_nb + 127) // 128) * 128)
    NQ = NA // 2

    if NA not in _CACHE:
        _CACHE[NA] = build(NA)
    nc = _CACHE[NA]

    wmaps = _prep_weights(*args)
    in_maps = _make_in_maps(hidden_states, idx, NA, wmaps)

    res = run_bass_kernel_spmd(nc, in_maps, core_ids=list(range(8)))

    out = np.zeros((B, S, D), np.float32)
    for c in range(8):
        b, half = c // 2, c % 2
        nb = len(idx[b])
        oT = res.results[c]["out"]  # (768, NQ)
        qtok = (np.arange(NQ) + half * NQ) % NA
        valid = qtok < nb
        out[b, idx[b][qtok[valid]]] = oT.T[valid]
    return out
